# revision 1
# baseline (speedup 1.0000x reference)
"""CompressedGPT2Attention on 8 TRN2 NeuronCores.

Sharding: core c = (batch b = c // 2, head-group g = c % 2) — data parallel on
B=4, tensor parallel over 16 heads (8 per group). Each core computes a partial
output [S, E] (its head-group's contribution + output_bias on g==0 cores);
host sums the two partials per batch.

Per-core pipeline (one flat scope; the in-order PE queue is software-
pipelined with a LAG so the ACT engine — exp is the bottleneck — never
starves):
  q~/k~ = 32*(q+b), 32*(k+b) [col, S] bf16: fp8e4m3 DoubleRow projections
          (numerically validated: q/k-path fp8 costs 9.5e-3 rel err)
  v     = hs @ wv + b in bf16 (fp8 v-path would cost 2.9e-2) with an
          exact-1.0 denominator column per head
  scores = k~.T @ q~ per (head, j-chunk, 1024-i-window); psum = 1024*s;
          causal mask = PE accumulate of tril(-1e9,-1)
  probs = exp(scores/8192) -> bf16 (ACT, the critical resource)
  attn: FLIPPED layout — out[i-part, r] = et_block.T @ v_head, free size 33
          per matmul (stationary et loads are free), [128, 8, 33] psum/head
  normalize: one DVE recip [128, 8] of per-partition denominators, DVE
          scalar-mult per i-subtile, PE transpose back to [r, i], DVE copy
          into attn8 [128, 2, S] bf16
  out = attn8.T @ wout + bias; window-0 outproj overlaps window-1
          attention; epilogue stagings split across DVE and ACT
"""

import numpy as np
import ml_dtypes
from contextlib import ExitStack

import concourse.bass as bass
import concourse.bacc as bacc
import concourse.tile as tile
import concourse.mybir as mybir
from concourse.bass_utils import run_bass_kernel_spmd

F32 = mybir.dt.float32
BF16 = mybir.dt.bfloat16
F8 = mybir.dt.float8e4
AF = mybir.ActivationFunctionType
ALU = mybir.AluOpType
DRM = mybir.MatmulPerfMode.DoubleRow

B, S, E = 4, 2048, 1024
H, HD, R = 16, 64, 32
HG = 8                 # heads per core
N_CORES = 8
WQ_SC = 32.0           # host scale on wq/wk (fp8 range); q~ = 32*(q+b)
EXP_SCALE = 1.0 / (8.0 * WQ_SC * WQ_SC)

LAG = 12               # units between score emission and attn emission

_PROGRAM_CACHE = {}


def _build_program():
    nc = bacc.Bacc("TRN2", target_bir_lowering=False, debug=False,
                   num_devices=N_CORES)

    hs8_d = nc.dram_tensor("hs8", [128, 4, 2, S], F8, kind="ExternalInput").ap()
    hsb_d = nc.dram_tensor("hsb", [128, 8, S], BF16, kind="ExternalInput").ap()
    wqk_d = nc.dram_tensor("wqk8", [128, 8, 2, 512], F8, kind="ExternalInput").ap()
    bqk_d = nc.dram_tensor("bqk", [128, 8], F32, kind="ExternalInput").ap()
    wv_d = nc.dram_tensor("wv", [128, 8, HG * 33], BF16, kind="ExternalInput").ap()
    wo_d = nc.dram_tensor("wo", [128, 2, E], BF16, kind="ExternalInput").ap()
    bvo_d = nc.dram_tensor("bvo", [1, HG * 33 + E], F32, kind="ExternalInput").ap()
    te_d = nc.dram_tensor("te", [128, 256], BF16, kind="ExternalInput").ap()
    out_d = nc.dram_tensor("out", [S, E], F32, kind="ExternalOutput").ap()

    with tile.TileContext(nc) as tc, ExitStack() as ctx:
        persist = ctx.enter_context(tc.tile_pool(name="persist", bufs=1))
        sc_pool = ctx.enter_context(tc.tile_pool(name="scps", bufs=2, space="PSUM"))
        at_pool = ctx.enter_context(tc.tile_pool(name="atps", bufs=2, space="PSUM"))
        tp_pool = ctx.enter_context(tc.tile_pool(name="tpps", bufs=1, space="PSUM"))
        op_pool = ctx.enter_context(tc.tile_pool(name="opps", bufs=1, space="PSUM"))
        exp_pool = ctx.enter_context(tc.tile_pool(name="exp", bufs=LAG + 4))
        nrm_pool = ctx.enter_context(tc.tile_pool(name="nrm", bufs=2))
        ob_pool = ctx.enter_context(tc.tile_pool(name="ob", bufs=4))

        q_sb = [persist.tile([128, S], BF16, name=f"q{m}", tag=f"q{m}") for m in range(4)]
        k_sb = [persist.tile([128, S], BF16, name=f"k{m}", tag=f"k{m}") for m in range(4)]
        v_sb = [persist.tile([128, 2, HG * 33], BF16, name=f"v{c}", tag=f"v{c}")
                for c in range(8)]
        attn8 = persist.tile([128, 2, S], BF16, name="attn8", tag="attn8")
        wo_sb = persist.tile([128, 2, E], BF16, name="wo", tag="wo")
        te_sb = persist.tile([128, 256], BF16, name="te", tag="te")
        tri_sb = te_sb[:, 0:128]
        eye_sb = te_sb[:, 128:256]
        bob_sb = persist.tile([1, E], BF16, name="bob", tag="bob")
        ones_b = persist.tile([1, 128], BF16, name="onesb", tag="onesb")
        nc.vector.memset(ones_b, 1.0)
        bout_bc = persist.tile([128, E], F32, name="bo_bc", tag="bo_bc")

        # ---- input DMAs: few, large, dependency-ordered (HWDGE issue is
        # serial at ~625ns per DMA; transfers serialize on the DMA engines)
        wqk_sb = persist.tile([128, 8, 2, 512], F8, name="wqk", tag="wqk")
        hs8_sb = persist.tile([128, 4, 2, S], F8, name="hs8", tag="hs8")
        hsb_sb = persist.tile([128, 8, S], BF16, name="hsb", tag="hsb")
        wv_sb = persist.tile([128, 8, HG * 33], BF16, name="wv", tag="wv")
        bqk_sb = persist.tile([128, 8], F32, name="bqk", tag="bqk")
        bvo_sb = persist.tile([1, HG * 33 + E], F32, name="bvo", tag="bvo")

        nc.sync.dma_start(out=wqk_sb[:, 0:4], in_=wqk_d[:, 0:4])
        nc.sync.dma_start(out=bvo_sb, in_=bvo_d)
        nc.sync.dma_start(out=hs8_sb[:, :, :, 0:1024], in_=hs8_d[:, :, :, 0:1024])
        nc.sync.dma_start(out=bqk_sb, in_=bqk_d)
        nc.sync.dma_start(out=hsb_sb[:, :, 0:256], in_=hsb_d[:, :, 0:256])
        nc.sync.dma_start(out=wqk_sb[:, 4:8], in_=wqk_d[:, 4:8])
        nc.sync.dma_start(out=te_sb, in_=te_d)
        nc.sync.dma_start(out=wv_sb, in_=wv_d)
        nc.sync.dma_start(out=hsb_sb[:, :, 256:1024], in_=hsb_d[:, :, 256:1024])
        nc.sync.dma_start(out=hs8_sb[:, :, :, 1024:S], in_=hs8_d[:, :, :, 1024:S])
        nc.sync.dma_start(out=hsb_sb[:, :, 1024:S], in_=hsb_d[:, :, 1024:S])
        nc.sync.dma_start(out=wo_sb, in_=wo_d)

        bva_sb = bvo_sb[:, 0:HG * 33]
        bov_sb = bvo_sb[:, HG * 33:HG * 33 + E]
        bva_bc = persist.tile([128, HG * 33], F32, name="bva_bc", tag="bva_bc")
        nc.gpsimd.partition_broadcast(bva_bc, bva_sb)
        nc.gpsimd.partition_broadcast(bout_bc, bov_sb)
        nc.vector.tensor_copy(out=bob_sb, in_=bov_sb)

        # ---- projection emitters (psum borrowed from op_pool's tag) ----
        def qk_proj(m, qoff, boff, dst, nbs):
            msl = slice(m * 128, m * 128 + 128)
            for nb in nbs:
                sl = slice(nb * 512, nb * 512 + 512)
                ps = op_pool.tile([128, 512], F32, name="ops", tag="ops")
                for P in range(4):
                    nc.tensor.matmul(ps, wqk_sb[:, qoff + P, :, msl],
                                     hs8_sb[:, P, :, sl],
                                     start=(P == 0), stop=(P == 3),
                                     perf_mode=DRM)
                nc.vector.tensor_scalar_add(
                    out=dst[m][:, sl], in0=ps,
                    scalar1=bqk_sb[:, boff + m:boff + m + 1])

        def v_proj(sc):
            ssl = slice(sc * 128, sc * 128 + 128)
            ps = op_pool.tile([128, 512], F32, name="ops", tag="ops")
            for P in range(8):
                nc.tensor.matmul(ps[:, 0:HG * 33], hsb_sb[:, P, ssl],
                                 wv_sb[:, P], start=(P == 0), stop=(P == 7))
            nc.vector.scalar_tensor_tensor(
                out=v_sb[sc // 2][:, sc % 2, :], in0=ps[:, 0:HG * 33],
                scalar=1.0, in1=bva_bc, op0=ALU.mult, op1=ALU.add)

        def outproj(it, eb):
            """Window-0 outproj, interleaved into window-1 (DVE staging)."""
            sl = slice(eb * 512, eb * 512 + 512)
            isl = slice(it * 128, it * 128 + 128)
            ps = op_pool.tile([128, 512], F32, name="ops", tag="ops")
            for t in range(2):
                nc.tensor.matmul(ps, attn8[:, t, isl], wo_sb[:, t, sl],
                                 start=(t == 0), stop=(t == 1))
            ot = ob_pool.tile([128, 512], F32, name="ot", tag="ot")
            nc.vector.scalar_tensor_tensor(
                out=ot, in0=ps, scalar=1.0, in1=bout_bc[:, sl],
                op0=ALU.mult, op1=ALU.add)
            nc.sync.dma_start(out=out_d[isl, sl], in_=ot)

        # prologue: only what window-0 head-0 needs right away
        qk_proj(0, 0, 0, q_sb, (0, 1))
        qk_proj(0, 4, 4, k_sb, (0, 1))

        # work interleaved into the unit stream: after unit index -> emitters,
        # spread wide so DVE staging never sits ahead of a normalize chain
        # that the PE queue is blocked on. w0 pairs: (h0,h1)=units 0-15,
        # (h2,h3)=16-31, (h4,h5)=32-47, (h6,h7)=48-63; w1 h starts 64+16h.
        # m-tile m serves heads 2m,2m+1; v pair c serves j-chunks 2c,2c+1.
        hooks = {}

        def hook(i, fn):
            hooks.setdefault(i, []).append(fn)

        for sc in range(8):          # v for window 0 (j-chunks 0..7)
            hook(sc, lambda sc=sc: v_proj(sc))
        for mi, base in ((1, 6), (2, 18), (3, 34)):   # q/k m1..m3 nb01
            for oi, (off, dst) in enumerate(((0, q_sb), (4, k_sb))):
                for nb in (0, 1):
                    hook(base + 2 * oi + nb,
                         lambda m=mi, off=off, d=dst, nb=nb:
                         qk_proj(m, off, off, d, (nb,)))
        for sc in range(8, 16):      # v for window 1 (j-chunks 8..15)
            hook(sc + 32, lambda sc=sc: v_proj(sc))
        for mi, base in ((0, 50), (1, 66), (2, 82), (3, 98)):  # nb23 for w1
            for oi, (off, dst) in enumerate(((0, q_sb), (4, k_sb))):
                for nb in (2, 3):
                    hook(base + 2 * oi + (nb - 2),
                         lambda m=mi, off=off, d=dst, nb=nb:
                         qk_proj(m, off, off, d, (nb,)))

        # unit = one (window, head, j-chunk) score block. Window-0 heads are
        # only 8 units (~5us of exp), too short to hide a head's serial
        # normalize chain — so interleave head PAIRS there (at_pool bufs=2
        # keeps both heads' psums live).
        units = []
        for hp in range(0, 8, 2):
            for jc in range(8):
                for h in (hp, hp + 1):
                    units.append((0, h, jc, jc * 128, True, jc == 7))
        for h in range(8):
            for jc in range(16):
                units.append((1, h, jc, max(jc * 128 - 1024, 0),
                              jc >= 8, jc == 15))

        at_tiles = {}
        et_tiles = {}

        def emit_attn(u):
            w, h, jc, lo, diag, last = u
            key = (w, h)
            if key not in at_tiles:
                at_tiles[key] = at_pool.tile([128, 8, 33], F32, name="at",
                                             tag="at")
            at = at_tiles[key]
            et = et_tiles.pop(u)
            vsl = v_sb[jc // 2][:, jc % 2, h * 33:h * 33 + 33]
            first = jc == 0
            lsubs = list(range(max(0, jc - 8 * w), 8))
            for li, l in enumerate(lsubs):
                # one psum group spans the whole head: start on the head's
                # first matmul, stop on its last (last unit, l == 7)
                nc.tensor.matmul(at[:, l, :], et[:, l * 128:l * 128 + 128],
                                 vsl, start=(first and li == 0),
                                 stop=(last and l == 7))
            if last:
                ibase = w * 1024
                roff = (h % 4) * 32
                rec = nrm_pool.tile([128, 8], F32, name="rec", tag="rec")
                nc.vector.reciprocal(out=rec, in_=at[:, :, 32])
                nsb = nrm_pool.tile([128, 8, 32], BF16, name="nsb", tag="nsb")
                tps = tp_pool.tile([32, 8, 128], BF16, name="tps", tag="tps")
                nc.vector.scalar_tensor_tensor(
                    out=nsb, in0=at[:, :, 0:32], scalar=1.0,
                    in1=rec.unsqueeze(2).broadcast_to([128, 8, 32]),
                    op0=ALU.mult, op1=ALU.mult)
                for l in range(8):
                    nc.tensor.matmul(tps[:, l, :], nsb[:, l, :], eye_sb,
                                     start=True, stop=True, is_transpose=True)
                nc.vector.tensor_copy(
                    out=attn8[roff:roff + 32, h // 4, ibase:ibase + 1024],
                    in_=tps.rearrange("p a b -> p (a b)"))
                del at_tiles[key]
                if w == 1:
                    outproj(h, 0)
                    outproj(h, 1)

        for i, u in enumerate(units):
            w, h, jc, lo, diag, last = u
            ibase = w * 1024
            pairm = h // 2
            dpart = slice((h % 2) * 64, (h % 2) * 64 + 64)
            jsl = slice(jc * 128, jc * 128 + 128)
            sp = sc_pool.tile([128, 1024], F32, name="sp", tag="sp")
            dnb = lo // 512 if diag else -1
            for nb in range(2):
                a = max(lo, nb * 512)
                bb = nb * 512 + 512
                if a >= bb:
                    continue
                nc.tensor.matmul(
                    sp[:, a:bb], k_sb[pairm][dpart, jsl],
                    q_sb[pairm][dpart, ibase + a:ibase + bb],
                    start=True, stop=(nb != dnb))
            if diag:
                nc.tensor.matmul(sp[:, lo:lo + 128], eye_sb, tri_sb,
                                 start=False, stop=True)
            et = exp_pool.tile([128, 1024], BF16, name="et", tag="et")
            et_tiles[u] = et
            nc.scalar.activation(out=et[:, lo:1024], in_=sp[:, lo:1024],
                                 func=AF.Exp, scale=EXP_SCALE)
            if i >= LAG:
                emit_attn(units[i - LAG])
            for fn in hooks.get(i, ()):
                fn()
        for u in units[-LAG:]:
            emit_attn(u)

        # epilogue: window-1 outproj as 16 (it, eb) halves; ACT-staged
        # halves pack two per [128,1024] sc slot, DVE-staged use the op slot
        halves = [(it, eb) for it in range(8, 16) for eb in range(2)]
        sc_cur = {}
        for idx, (it, eb) in enumerate(halves):
            isl = slice(it * 128, it * 128 + 128)
            sl = slice(eb * 512, eb * 512 + 512)
            use_act = idx % 2 == 0
            if use_act:
                half = (idx // 2) % 2
                if half == 0:
                    sc_cur[0] = sc_pool.tile([128, 1024], F32, name="sp",
                                             tag="sp")
                ps = sc_cur[0][:, half * 512:half * 512 + 512]
            else:
                ps = op_pool.tile([128, 512], F32, name="ops", tag="ops")
            for t in range(2):
                nc.tensor.matmul(ps, attn8[:, t, isl], wo_sb[:, t, sl],
                                 start=(t == 0), stop=False)
            nc.tensor.matmul(ps, ones_b, bob_sb[:, sl], start=False, stop=True)
            ot = ob_pool.tile([128, 512], F32, name="ot", tag="ot")
            if use_act:
                nc.scalar.activation(out=ot, in_=ps, func=AF.Copy,
                                     bias=0.0, scale=1.0)
            else:
                nc.vector.tensor_copy(out=ot, in_=ps)
            nc.sync.dma_start(out=out_d[isl, sl], in_=ot)

    nc.compile()
    return nc


def _get_program():
    if "nc" not in _PROGRAM_CACHE:
        _PROGRAM_CACHE["nc"] = _build_program()
    return _PROGRAM_CACHE["nc"]


def _bf(a):
    return np.asarray(a, dtype=np.float32).astype(ml_dtypes.bfloat16)


def _f8(a):
    return np.asarray(a, dtype=np.float32).astype(ml_dtypes.float8_e4m3)


def _pack_rows(a2d, inner):
    """[1024, inner] -> [128, 8, inner] ([p, ec, :] = a2d[ec*128+p])."""
    return np.ascontiguousarray(a2d.reshape(8, 128, inner).swapaxes(0, 1))


def _pack_pairs(a2d, inner):
    """[1024, inner] -> [128, npair, 2, inner] fp8 DoubleRow pair layout."""
    n = a2d.shape[0] // 256
    return _f8(np.ascontiguousarray(
        a2d.reshape(n, 2, 128, inner).transpose(2, 0, 1, 3)))


def kernel(hidden_states, q_weight, q_bias, k_weight, k_bias,
           low_rank_value_weight, low_rank_value_bias,
           low_rank_output_weight, output_bias):
    hidden_states = np.asarray(hidden_states, dtype=np.float32)
    q_weight = np.asarray(q_weight, dtype=np.float32)
    q_bias = np.asarray(q_bias, dtype=np.float32)
    k_weight = np.asarray(k_weight, dtype=np.float32)
    k_bias = np.asarray(k_bias, dtype=np.float32)
    wv_full = np.asarray(low_rank_value_weight, dtype=np.float32)
    bv_full = np.asarray(low_rank_value_bias, dtype=np.float32)
    wout_full = np.asarray(low_rank_output_weight, dtype=np.float32)
    output_bias = np.asarray(output_bias, dtype=np.float32)

    te = np.zeros((128, 256), dtype=np.float32)
    te[:, 0:128] = np.tril(np.full((128, 128), -1e9, dtype=np.float32), k=-1)
    te[:, 128:256] = np.eye(128, dtype=np.float32)
    te = te.astype(ml_dtypes.bfloat16)

    in_maps = []
    for c in range(N_CORES):
        b, g = c // 2, c % 2
        hs_t = np.ascontiguousarray(hidden_states[b].T)          # [E, S]
        cols = slice(g * 512, (g + 1) * 512)
        vcols = slice(g * 256, (g + 1) * 256)

        wv_aug = np.zeros((E, HG * 33), dtype=np.float32)
        wv_g = wv_full[:, vcols].reshape(E, HG, R)
        bv_g = bv_full[vcols].reshape(HG, R)
        for h in range(HG):
            wv_aug[:, h * 33:h * 33 + 32] = wv_g[:, h, :]

        bvo = np.zeros((1, HG * 33 + E), dtype=np.float32)
        for h in range(HG):
            bvo[0, h * 33:h * 33 + 32] = bv_g[h]
            bvo[0, h * 33 + 32] = 1.0
        bout = (output_bias if g == 0 else np.zeros_like(output_bias))
        bvo[0, HG * 33:] = bout

        wqk = np.empty((128, 8, 2, 512), dtype=ml_dtypes.float8_e4m3)
        wqk[:, 0:4] = _pack_pairs(WQ_SC * q_weight[:, cols], 512)
        wqk[:, 4:8] = _pack_pairs(WQ_SC * k_weight[:, cols], 512)
        bqk = np.empty((128, 8), dtype=np.float32)
        bqk[:, 0:4] = (WQ_SC * q_bias[cols]).reshape(4, 128).T
        bqk[:, 4:8] = (WQ_SC * k_bias[cols]).reshape(4, 128).T

        in_maps.append({
            "hs8": _pack_pairs(hs_t, S),
            "hsb": _bf(_pack_rows(hs_t, S)),
            "wqk8": wqk,
            "bqk": bqk,
            "wv": _bf(_pack_rows(wv_aug, HG * 33)),
            "wo": _bf(wout_full[vcols, :].reshape(2, 128, E).swapaxes(0, 1)),
            "bvo": bvo,
            "te": te,
        })

    nc = _get_program()
    res = run_bass_kernel_spmd(nc, in_maps, list(range(N_CORES)))
    out = np.empty((B, S, E), dtype=np.float32)
    for b in range(B):
        out[b] = res.results[2 * b]["out"] + res.results[2 * b + 1]["out"]
    return out

